# revision 21
# baseline (speedup 1.0000x reference)
"""Cross-modal attention TRN2 kernel (v4).

Problem: B=4, N=2048, IN_DIM=DIM=1024, HEADS=8, D_HEAD=128, scale=DIM**-0.5.
  q = x_a @ W_q.T ; k,v = split(x_b @ W_kv.T) ; per-head softmax(q k^T/32) v ;
  out = merge_heads @ W_out.T + b_out

Sharding over 8 cores: core c -> batch b=c//2, head-half hh=c%2 (4 heads,
512 of DIM).  W_q/W_kv column-sharded, W_out row-sharded (Megatron); each
core emits a partial output projection y_cT = (W_out[:, slice] @ O_half)
of shape [DIM, N]; host sums the two head-half partials per batch, adds
b_out, transposes back.

Evolution (HW exec, core 0):
  v1 fp32r baseline                            410us  (PE 80% busy)
  v2 bf16 + fast reciprocal + pair-summed dens 315us
  v3 gpsimd warmup + fused weight DMA +
     deferred normalize muls                   302us
  v4 host-pre-shuffled contiguous DRAM layouts (every DMA is one
     contiguous run per partition), interleaved matmul stationaries so
     LDWEIGHTS hides under the previous matmul, per-chunk-contiguous
     output buffer.

Device layout: everything transposed ([feature, token]) so all matmuls
contract over the partition dim.  All matmul operands bf16 (full PE rate,
half DMA/SBUF); PSUM fp32.  Softmax uses no max-subtraction (|s|*scale
< ~1 by construction); exp tiles are pair-summed on DVE and the pair sums
matmul'd against ones into [1,512] PSUM rows; reciprocal_approx_fast +
gpsimd partition_broadcast normalize O after each head block, with the
muls deferred into the next block so the DVE never queues behind gpsimd.
"""

import numpy as np

B, N, IN_DIM, DIM, HEADS = 4, 2048, 1024, 1024, 8
D_HEAD = DIM // HEADS          # 128
SCALE = DIM ** -0.5            # 1/32
NCORES = 8
HH = HEADS // 2                # 4 heads per core
DVC = HH * D_HEAD              # 512 dv per core
P = 128
KT = IN_DIM // P               # 8 contraction tiles
NJT = N // P                   # 16 j tiles
NIB = N // 512                 # 4 i-blocks of 512
IB2 = N // 1024                # 2 i-blocks of 1024
BW = 512                       # phase-1 streaming block width
NB = N // BW                   # 4 blocks

_TRACE = False
REPS = 1
LAST_EXEC_NS = None
LAST_RESULTS = None


def _build_nc(reps=1):
    import concourse.tile as tile
    from concourse import bacc, mybir

    f32 = mybir.dt.float32
    bf16 = mybir.dt.bfloat16
    Exp = mybir.ActivationFunctionType.Exp

    nc = bacc.Bacc("TRN2", debug=False, num_devices=NCORES)

    # host-pre-shuffled layouts: row (blk*128+p) holds partition p's
    # contiguous data for that block -> every DMA is one run per partition
    xaS = nc.dram_tensor("xaS", [NB * P, KT * BW], bf16, kind="ExternalInput").ap()
    xbS = nc.dram_tensor("xbS", [NB * P, KT * BW], bf16, kind="ExternalInput").ap()
    wqS = nc.dram_tensor("wqS", [P, KT * DVC], bf16, kind="ExternalInput").ap()
    wkS = nc.dram_tensor("wkS", [P, KT * DVC], bf16, kind="ExternalInput").ap()
    wvS = nc.dram_tensor("wvS", [P, KT * DVC], bf16, kind="ExternalInput").ap()
    woS = nc.dram_tensor("woS", [P, HH * DIM], bf16, kind="ExternalInput").ap()
    # eh[:, 2h:2h+2] is all-ones in column h: a den matmul with that
    # stationary adds the tile's exp-sums into PSUM row h only
    eh_d = nc.dram_tensor("eh", [P, 4], bf16, kind="ExternalInput").ap()
    # y chunk (ib, e8) = rows (ib*8+e8)*128..+128, fully contiguous
    yS = nc.dram_tensor("yS", [NIB * DIM, 512], bf16, kind="ExternalOutput").ap()

    with tile.TileContext(nc) as tc:
      for _rep in range(reps):
        with tc.tile_pool(name="persist", bufs=1) as persist:
            qT_sb = persist.tile([P, HH, N], bf16)      # [d%128, head, i]
            kT_sb = persist.tile([P, HH, N], bf16)      # [d%128, head, j]
            v_sb = persist.tile([P, NJT, DVC], bf16)    # [j%128, jt, dv]
            oT_ts = [[persist.tile([P, 1024], bf16, tag=f"o{h}_{bb}",
                                   name=f"o{h}_{bb}")
                      for bb in range(IB2)] for h in range(HH)]
            eh_sb = persist.tile([P, 4], bf16)
            wo_sb = persist.tile([P, HH * DIM], bf16)
            warm = persist.tile([P, 1], f32)
            warmb = persist.tile([P, 1], bf16)
            nc.sync.dma_start(out=eh_sb, in_=eh_d)
            # warm the ACT exp table set and the GPSIMD broadcast ucode
            # (first PartitionBroadcast otherwise pays a ~7us library load
            # mid-attention) while phase-1 DMA streams in
            nc.scalar.activation(warm, eh_sb[:, :1], Exp)
            nc.gpsimd.partition_broadcast(warmb, eh_sb[:1, :1])

            # ---------------- phase 1: projections (K,V first) --------
            with tc.tile_pool(name="wpool", bufs=1) as wpool, \
                 tc.tile_pool(name="xblk", bufs=3) as xblk, \
                 tc.tile_pool(name="psum1", bufs=4, space="PSUM") as psum1:
                wq_sb = wpool.tile([P, KT * DVC], bf16, tag="wq", name="wq")
                wk_sb = wpool.tile([P, KT * DVC], bf16, tag="wk", name="wk")
                wv_sb = wpool.tile([P, KT * DVC], bf16, tag="wv", name="wv")

                nc.scalar.dma_start(out=wk_sb, in_=wkS)

                for jb in range(NB):
                    xb_blk = xblk.tile([P, KT * BW], bf16, tag="xblk")
                    nc.sync.dma_start(
                        out=xb_blk, in_=xbS[jb * P:(jb + 1) * P, :])
                    if jb == 0:
                        nc.scalar.dma_start(out=wv_sb, in_=wvS)
                    for dt in range(HH):
                        ps = psum1.tile([P, BW], f32, tag="ps1")
                        for kt in range(KT):
                            nc.tensor.matmul(
                                ps,
                                wk_sb[:, kt * DVC + dt * P:
                                      kt * DVC + (dt + 1) * P],
                                xb_blk[:, kt * BW:(kt + 1) * BW],
                                start=(kt == 0), stop=(kt == KT - 1))
                        nc.vector.tensor_copy(
                            kT_sb[:, dt, jb * BW:(jb + 1) * BW], ps)
                    for j2 in range(BW // P):
                        jt = jb * (BW // P) + j2
                        ps = psum1.tile([P, DVC], f32, tag="psv")
                        for kt in range(KT):
                            nc.tensor.matmul(
                                ps,
                                xb_blk[:, kt * BW + j2 * P:
                                       kt * BW + (j2 + 1) * P],
                                wv_sb[:, kt * DVC:(kt + 1) * DVC],
                                start=(kt == 0), stop=(kt == KT - 1))
                        nc.vector.tensor_copy(v_sb[:, jt, :], ps)
                    if jb == 0:
                        nc.scalar.dma_start(out=wq_sb, in_=wqS)

                for ib in range(NB):
                    xa_blk = xblk.tile([P, KT * BW], bf16, tag="xblk")
                    nc.sync.dma_start(
                        out=xa_blk, in_=xaS[ib * P:(ib + 1) * P, :])
                    for dt in range(HH):
                        ps = psum1.tile([P, BW], f32, tag="ps1")
                        for kt in range(KT):
                            nc.tensor.matmul(
                                ps,
                                wq_sb[:, kt * DVC + dt * P:
                                      kt * DVC + (dt + 1) * P],
                                xa_blk[:, kt * BW:(kt + 1) * BW],
                                start=(kt == 0), stop=(kt == KT - 1))
                        nc.vector.tensor_copy(
                            qT_sb[:, dt, ib * BW:(ib + 1) * BW], ps)
                    if ib == NB - 1:
                        # W_out arrives during the attention phase
                        nc.scalar.dma_start(out=wo_sb, in_=woS)

            # ---------------- phase 2: attention ----------------
            with tc.tile_pool(name="expp", bufs=8) as expp, \
                 tc.tile_pool(name="esum", bufs=3) as esum, \
                 tc.tile_pool(name="bcp", bufs=4) as bcp, \
                 tc.tile_pool(name="rcp", bufs=2) as rcp, \
                 tc.tile_pool(name="ystage", bufs=4) as ystage, \
                 tc.tile_pool(name="dotsp", bufs=2, space="PSUM") as dotsp, \
                 tc.tile_pool(name="avp", bufs=1, space="PSUM") as avp, \
                 tc.tile_pool(name="auxp", bufs=1, space="PSUM") as auxp, \
                 tc.tile_pool(name="denp", bufs=1, space="PSUM") as denp:
                LAG = 2   # PV/den trail dots/exp so the PE never waits on
                          # the ACT exp of the current tile
                pending = []  # deferred normalize muls: (osl, sl, bc)

                # first half of the output projection (bb=0) is interleaved
                # into the (ib=1, h>=1) attention blocks: the PE has slack in
                # the ACT-bound steady state, and this pulls ~14us out of the
                # serial phase-3 tail.  yops = flat (chain, dt) work items.
                yops = [(c, dt) for c in range(16) for dt in range(HH)]
                ystate = {}

                def emit_yop():
                    c, dt = yops.pop(0)
                    ib3, e8 = divmod(c, 8)
                    bb, half = divmod(ib3, 2)
                    if dt == 0:
                        ystate['ps'] = auxp.tile([P, 512], f32, tag="aux",
                                                 name=f"yaux{c}")
                    nc.tensor.matmul(
                        ystate['ps'],
                        wo_sb[:, dt * DIM + e8 * P:dt * DIM + (e8 + 1) * P],
                        oT_ts[dt][bb][:, half * 512:(half + 1) * 512],
                        start=(dt == 0), stop=(dt == HH - 1))
                    if dt == HH - 1:
                        ys = ystage.tile([P, 512], bf16, tag="ys")
                        nc.vector.tensor_copy(ys, ystate['ps'])
                        nc.sync.dma_start(
                            out=yS[(ib3 * 8 + e8) * P:
                                   (ib3 * 8 + e8 + 1) * P, :],
                            in_=ys)

                for ib in range(IB2):
                    i0 = ib * 1024
                    for h in range(HH):
                        po = avp.tile([P, 1024], f32)
                        pd = denp.tile([2, BW], f32, tag="pd",
                                       name=f"pd_{h}_{ib}")
                        ets = {}
                        sums = {}
                        for step in range(NJT + LAG + 2):
                            if step == 6:
                                # previous block's normalize muls go here so
                                # the DVE never queues behind the gpsimd
                                # broadcasts at a block boundary
                                for osl_p, sl_p, bc_p in pending:
                                    nc.vector.tensor_mul(
                                        osl_p[:, sl_p], osl_p[:, sl_p], bc_p)
                                pending = []
                            if ib == 1 and h >= 1 and yops:
                                emit_yop()
                                if step < 6 and yops:
                                    emit_yop()
                            jt = step
                            jd = step - LAG
                            # interleave dots/PV so consecutive matmuls use
                            # different stationaries and each LDWEIGHTS hides
                            # under the preceding matmul
                            et_d = ets[jd] if 0 <= jd < NJT else None
                            ps = None
                            if jt < NJT:
                                ps = dotsp.tile([P, 1024], f32, tag="ps")
                                k_l = kT_sb[:, h, jt * P:(jt + 1) * P]
                            v_l = v_sb[:, jd, h * P:(h + 1) * P] \
                                if et_d is not None else None
                            for hf in range(2):
                                sl = slice(hf * 512, (hf + 1) * 512)
                                if ps is not None:
                                    nc.tensor.matmul(
                                        ps[:, sl],
                                        k_l,
                                        qT_sb[:, h, i0 + hf * 512:
                                              i0 + (hf + 1) * 512],
                                        start=True, stop=True)
                                if et_d is not None:
                                    nc.tensor.matmul(
                                        po[:, sl], v_l, et_d[:, sl],
                                        start=(jd == 0), stop=(jd == NJT - 1))
                            if ps is not None:
                                et = expp.tile([P, 1024], bf16, tag="exp")
                                nc.scalar.activation(et, ps, Exp, scale=SCALE)
                                ets[jt] = et
                            if et_d is not None and jd % 2 == 1:
                                es = esum.tile([P, 1024], bf16, tag="es")
                                nc.vector.tensor_add(
                                    es, ets.pop(jd - 1), ets.pop(jd))
                                sums[(jd - 1) // 2] = es
                            # denominator matmuls trail the pair-sum by one
                            # step so the PE never waits on the DVE add
                            je = step - LAG - 1
                            if je >= 1 and je % 2 == 1:
                                pk = (je - 1) // 2
                                es = sums.pop(pk)
                                for hf in range(2):
                                    # one-hot stationary -> row hf of pd;
                                    # both halves share one PSUM bank
                                    nc.tensor.matmul(
                                        pd, eh_sb[:, 2 * hf:2 * hf + 2],
                                        es[:, hf * 512:(hf + 1) * 512],
                                        start=(pk == 0 and hf == 0),
                                        stop=(pk == NJT // 2 - 1 and hf == 1))
                        # drain the PV accumulator to SBUF right away so the
                        # PSUM bank frees for the next block; reciprocal +
                        # broadcast now, the muls during the next block.
                        osl = oT_ts[h][ib]
                        nc.vector.tensor_copy(osl, po)
                        rc = rcp.tile([2, BW], f32, tag="rc")
                        nc.vector.reciprocal_approx_fast(out=rc, in_=pd)
                        # gpsimd broadcast reads partition 0 only; hop row 1
                        # down via a tiny SBUF->SBUF DMA
                        rc1 = rcp.tile([1, BW], f32, tag="rc1")
                        nc.sync.dma_start(out=rc1, in_=rc[1:2, :])
                        for hf in range(2):
                            bc = bcp.tile([P, BW], f32, tag="bc")
                            nc.gpsimd.partition_broadcast(
                                bc, rc[0:1, :] if hf == 0 else rc1)
                            sl = slice(hf * 512, (hf + 1) * 512)
                            pending.append((osl, sl, bc))
                for osl_p, sl_p, bc_p in pending:
                    nc.vector.tensor_mul(osl_p[:, sl_p], osl_p[:, sl_p], bc_p)
                pending = []

                # ---------------- phase 3: output projection (bb=1) --------
                # y-psum tiles share the dots pool slots (tag "ps"), which
                # free as the exp of the final j-tiles completes
                for ib in range(2, NIB):
                    bb, half = divmod(ib, 2)
                    for e8 in range(DIM // P):
                        ps = dotsp.tile([P, 1024], f32, tag="ps")
                        for dt in range(HH):
                            nc.tensor.matmul(
                                ps[:, :512],
                                wo_sb[:, dt * DIM + e8 * P:
                                      dt * DIM + (e8 + 1) * P],
                                oT_ts[dt][bb][:, half * 512:(half + 1) * 512],
                                start=(dt == 0), stop=(dt == HH - 1))
                        ys = ystage.tile([P, 512], bf16, tag="ys")
                        nc.scalar.copy(ys, ps[:, :512])
                        nc.sync.dma_start(
                            out=yS[(ib * 8 + e8) * P:
                                   (ib * 8 + e8 + 1) * P, :],
                            in_=ys)

    nc.compile()
    return nc


_nc_by_reps = {}


def _get_nc(reps=1):
    if reps not in _nc_by_reps:
        _nc_by_reps[reps] = _build_nc(reps)
    return _nc_by_reps[reps]


def _shuf_x(xT_bf):
    # xT [IN_DIM, N] -> [blk*128+p, kt*512+ii] with
    # out[blk*128+p, kt*512+ii] = xT[kt*128+p, blk*512+ii]
    a = xT_bf.reshape(KT, P, NB, BW)          # [kt, p, blk, ii]
    return np.ascontiguousarray(
        a.transpose(2, 1, 0, 3).reshape(NB * P, KT * BW))


def _shuf_w(wT_bf):
    # wT [IN_DIM, E] -> [p, kt*E+e] with out[p, kt*E+e] = wT[kt*128+p, e]
    e = wT_bf.shape[1]
    a = wT_bf.reshape(KT, P, e)
    return np.ascontiguousarray(a.transpose(1, 0, 2).reshape(P, KT * e))


def _make_in_maps(x_a, x_b, W_q, W_kv, W_out):
    import ml_dtypes

    bf = ml_dtypes.bfloat16
    xaS = [_shuf_x(np.ascontiguousarray(x_a[b].T).astype(bf))
           for b in range(B)]
    xbS = [_shuf_x(np.ascontiguousarray(x_b[b].T).astype(bf))
           for b in range(B)]
    eh = np.zeros((P, 4), dtype=bf)
    eh[:, 0] = 1   # hf0 stationary: column 0 all-ones
    eh[:, 3] = 1   # hf1 stationary: column 1 all-ones
    in_maps = []
    for c in range(NCORES):
        b, hh = divmod(c, 2)
        hs = hh * DVC
        # woS[p, dt*DIM+e] = W_out[e, hs+dt*128+p]
        wo = np.ascontiguousarray(W_out[:, hs:hs + DVC].T).astype(bf)
        woS = np.ascontiguousarray(
            wo.reshape(HH, P, DIM).transpose(1, 0, 2).reshape(P, HH * DIM))
        in_maps.append({
            "xaS": xaS[b],
            "xbS": xbS[b],
            "wqS": _shuf_w(
                np.ascontiguousarray(W_q[hs:hs + DVC].T).astype(bf)),
            "wkS": _shuf_w(
                np.ascontiguousarray(W_kv[hs:hs + DVC].T).astype(bf)),
            "wvS": _shuf_w(np.ascontiguousarray(
                W_kv[DIM + hs:DIM + hs + DVC].T).astype(bf)),
            "woS": woS,
            "eh": eh,
        })
    return in_maps


def kernel(x_a, x_b, W_q, W_kv, W_out, b_out):
    global LAST_EXEC_NS, LAST_RESULTS
    from concourse import bass_utils

    x_a = np.asarray(x_a, dtype=np.float32)
    x_b = np.asarray(x_b, dtype=np.float32)
    W_q = np.asarray(W_q, dtype=np.float32)
    W_kv = np.asarray(W_kv, dtype=np.float32)
    W_out = np.asarray(W_out, dtype=np.float32)
    b_out = np.asarray(b_out, dtype=np.float32)

    nc = _get_nc(REPS)
    in_maps = _make_in_maps(x_a, x_b, W_q, W_kv, W_out)

    res = bass_utils.run_bass_kernel_spmd(
        nc, in_maps, core_ids=list(range(NCORES)), trace=_TRACE)
    LAST_EXEC_NS = res.exec_time_ns
    LAST_RESULTS = res

    out = np.empty((B, N, DIM), dtype=np.float32)
    for b in range(B):
        # yS [(ib*8+e8)*128+p, ii] -> yT [e8*128+p, ib*512+ii]
        acc = (res.results[2 * b]["yS"].astype(np.float32)
               + res.results[2 * b + 1]["yS"].astype(np.float32))
        yT = acc.reshape(NIB, DIM, 512).transpose(1, 0, 2).reshape(DIM, N)
        out[b] = yT.T + b_out
    return out


def bench(inputs, reps_pair=(1, 9), iters=5):
    """Measure on-device time per kernel body via rep-delta wall timing."""
    import time
    from concourse import bass_utils
    ins = {k: np.asarray(v, dtype=np.float32) for k, v in inputs.items()
           if k != "b_out"}
    in_maps = _make_in_maps(ins["x_a"], ins["x_b"], ins["W_q"], ins["W_kv"],
                            ins["W_out"])
    walls = {}
    for reps in reps_pair:
        nc = _get_nc(reps)
        bass_utils.run_bass_kernel_spmd(nc, in_maps, core_ids=list(range(NCORES)))
        ts = []
        for _ in range(iters):
            t0 = time.perf_counter()
            bass_utils.run_bass_kernel_spmd(nc, in_maps,
                                            core_ids=list(range(NCORES)))
            ts.append(time.perf_counter() - t0)
        walls[reps] = min(ts)
        print(f"reps={reps}: wall min={walls[reps]*1e3:.2f} ms  all={[f'{t*1e3:.1f}' for t in ts]}")
    r0, r1 = reps_pair
    ns = (walls[r1] - walls[r0]) / (r1 - r0) * 1e9
    print(f"per-body device time: {ns:.0f} ns")
    return ns
